# revision 40
# baseline (speedup 1.0000x reference)
"""Trainium2 Bass kernel for the nn_MultiHeadAttention problem.

Data-parallel over batch: each of the 8 NeuronCores processes one batch
element independently (no collectives).

Mask compaction: the host gathers only the valid query/key positions
(QMask/KMask true), padded to a multiple of 128, and scatters the
output back (masked query rows are exactly zero in the reference).
With ~50% random masks this cuts the attention work ~4x.  The tile
counts (ntq, ntk) are chosen from the actual masks at kernel() time and
a bass program is compiled per shape, so any mask density works.

Per-core dataflow (E=1024, H=16, D=64; Lq=ntq*128 queries, Lk=ntk*128
keys after compaction; e-chunks of 128 = 2 heads):

  proj:  one blockdiag weight per chunk projects q and k in a single
         fused rhs ([QTc | KTc]), both heads at once; v2 per k-tile
         gets a validity "ones" column per head.
  scores: s[k,q] psum = kT_h_slice.T @ qT_h (bf16); P = exp(s/8), one
         ACT op per [128,Lq] tile -> bf16.  No max subtraction
         (|s|/8 <~ 13); masked/pad keys have v-rows and ones-column
         zeroed, reproducing masked_fill+softmax exactly.
  PV:    out[65,q] psum = sum_k v2_slice.T @ P_slice (bf16); row 64 is
         the softmax denominator.  Fast psum evacuation on DVE (denom
         row -> dstack via partition-64 staging + DMA shuffle, rows
         0:64 -> ct unnormalized); reciprocals in three batches off the
         critical path, then DRAM-bounce broadcast + one DVE multiply
         per head normalizes ct in place.
  final: split output projection after the main loop: part A (chunks
         0-6, already normalized) overlaps the last normalize chain;
         part B adds chunk 7 via ysum (SBUF f32) and writes Y.
"""

import math
import os
import sys

import numpy as np

try:
    import concourse  # noqa: F401
except ImportError:  # pragma: no cover
    for _p in ("/opt/trn_rl_repo", os.path.expanduser("~/.axon_site/_ro/trn_rl_repo")):
        if os.path.isdir(_p) and _p not in sys.path:
            sys.path.insert(0, _p)

import ml_dtypes

import concourse.bass as bass
import concourse.tile as tile
from concourse import bacc, mybir

B, L, E, H, D = 8, 1024, 1024, 16, 64
P = 128          # partitions
NCH = E // P     # 8 e-chunks (2 heads each)
F32 = mybir.dt.float32
BF16 = mybir.dt.bfloat16

# normalize batches: (head range start, end, after-chunk)
NORM_BATCHES = [(0, 8, 3), (8, 14, 6), (14, 16, 7)]


def _chunks(n, step=512):
    return [(s, min(s + step, n)) for s in range(0, n, step)]


def build_bass(ntq, ntk):
    Lq, Lk = ntq * P, ntk * P
    nc = bacc.Bacc(None, target_bir_lowering=False, debug=False)

    QT = nc.declare_dram_parameter("QT", [E, Lq], BF16, isOutput=False)
    KT = nc.declare_dram_parameter("KT", [E, Lk], BF16, isOutput=False)
    VT = nc.declare_dram_parameter("VT", [E, Lk], BF16, isOutput=False)
    W2 = nc.declare_dram_parameter("W2", [P, NCH, P], BF16, isOutput=False)
    OB = nc.declare_dram_parameter("OB", [E, E], BF16, isOutput=False)
    KM = nc.declare_dram_parameter("KM", [P, ntk], F32, isOutput=False)
    Y = nc.declare_dram_parameter("Y", [Lq, E], F32, isOutput=True)
    rbounce = nc.dram_tensor("rbounce", [H, Lq], BF16)

    with tile.TileContext(nc) as tc:
        with (
            tc.tile_pool(name="singles", bufs=1) as singles,
            tc.tile_pool(name="qkT", bufs=2) as qkT,
            tc.tile_pool(name="vaug", bufs=2) as vaug,
            tc.tile_pool(name="ppool", bufs=2) as ppool,
            tc.tile_pool(name="ystage", bufs=2) as ystage,
            tc.tile_pool(name="bcpool", bufs=3) as bcpool,
            tc.tile_pool(name="dtpool", bufs=2) as dtpool,
            tc.tile_pool(name="psbig", bufs=2, space="PSUM") as psbig,
            tc.tile_pool(name="pspv", bufs=1, space="PSUM") as pspv,
            tc.tile_pool(name="pssmall", bufs=2, space="PSUM") as pssmall,
        ):
            # --- persistent SBUF tensors -------------------------------
            qts = singles.tile([P, NCH, Lq], BF16)
            kts = singles.tile([P, NCH, Lk], BF16)
            vts = singles.tile([P, NCH, Lk], BF16)
            obs = singles.tile([P, NCH, E], BF16)
            w2s = singles.tile([P, NCH, P], BF16)
            kms = singles.tile([P, ntk], F32)
            ct = singles.tile([P, NCH, Lq], BF16)
            ysum = singles.tile([P, ntq, E], F32)
            dstacks = []
            rstacks = []
            for bi, (h0, h1, _) in enumerate(NORM_BATCHES):
                ds = singles.tile([(h1 - h0) * ntq, P], F32, tag=f"ds{bi}")
                rs = singles.tile([(h1 - h0) * ntq, P], BF16, tag=f"rs{bi}")
                dstacks.append(ds)
                rstacks.append(rs)

            # --- input DMAs (small/consts first, then per-chunk) -------
            nc.gpsimd.dma_start(out=w2s[:], in_=W2[:])
            nc.gpsimd.dma_start(out=kms[:], in_=KM[:])
            # PE warmup: ~8us of dummy matmuls while input DMAs land, so
            # the HAM clock gate opens before real work starts
            warm = singles.tile([P, 512], BF16)
            nc.vector.memset(warm[:], 0.0)
            for wi in range(16):
                wps = pssmall.tile([P, 512], F32, tag="small")
                nc.tensor.matmul(out=wps[:], lhsT=warm[:, 0:128], rhs=warm[:],
                                 start=True, stop=True)
            for c in range(NCH):
                nc.sync.dma_start(out=qts[:, c, :], in_=QT[c * P:(c + 1) * P, :])
                nc.sync.dma_start(out=kts[:, c, :], in_=KT[c * P:(c + 1) * P, :])
                nc.sync.dma_start(out=vts[:, c, :], in_=VT[c * P:(c + 1) * P, :])
            for c in range(NCH):
                nc.sync.dma_start(out=obs[:, c, :], in_=OB[c * P:(c + 1) * P, :])

            def normalize_batch(bi):
                h0, h1, _ = NORM_BATCHES[bi]
                with nc.allow_low_precision(reason="softmax recip bf16"):
                    nc.vector.reciprocal(out=rstacks[bi][:], in_=dstacks[bi][:])
                nc.gpsimd.dma_start(out=rbounce[h0:h1, :], in_=rstacks[bi][:])
                for h in range(h0, h1):
                    c, hf = h // 2, h % 2
                    bcs = bcpool.tile([P, Lq], BF16)
                    src = rbounce[h:h + 1, :]
                    bc_in = bass.AP(
                        tensor=src.tensor, offset=src.offset,
                        ap=[[0, P], list(src.ap[-1])])
                    nc.gpsimd.dma_start(out=bcs[:], in_=bc_in)
                    sl = ct[64 * hf:64 * hf + 64, c, :]
                    nc.vector.tensor_mul(sl, sl, bcs[64 * hf:64 * hf + 64, :])

            def final_mms(t, yps, crange):
                for c in crange:
                    for eh in range(2):
                        nc.tensor.matmul(
                            out=yps[:, 512 * eh:512 * (eh + 1)],
                            lhsT=ct[:, c, t * P:(t + 1) * P],
                            rhs=obs[:, c, 512 * eh:512 * (eh + 1)],
                            start=(c == crange[0]), stop=(c == crange[-1]),
                        )

            # --- main loop over e-chunks (2 heads each) ----------------
            for c in range(NCH):
                # fused q/k projection for both heads of this chunk
                qkt2 = qkT.tile([P, Lq + Lk], BF16, tag="qkt2")
                qt2 = qkt2[:, 0:Lq]
                kt2 = qkt2[:, Lq:Lq + Lk]
                for s0, s1 in _chunks(Lq + Lk):
                    ps = pssmall.tile([P, 512], F32, tag="small")
                    # fused rhs: columns [0,Lq) from qts, [Lq,Lq+Lk) from kts
                    if s1 <= Lq:
                        rhs = qts[:, c, s0:s1]
                    elif s0 >= Lq:
                        rhs = kts[:, c, s0 - Lq:s1 - Lq]
                    else:
                        rhs = None
                    if rhs is not None:
                        nc.tensor.matmul(
                            out=ps[:, 0:s1 - s0], lhsT=w2s[:, c, :], rhs=rhs,
                            start=True, stop=True)
                        nc.scalar.copy(qkt2[:, s0:s1], ps[:, 0:s1 - s0])
                    else:
                        mid = Lq - s0
                        nc.tensor.matmul(
                            out=ps[:, 0:mid], lhsT=w2s[:, c, :],
                            rhs=qts[:, c, s0:Lq], start=True, stop=True)
                        nc.tensor.matmul(
                            out=ps[:, mid:s1 - s0], lhsT=w2s[:, c, :],
                            rhs=kts[:, c, 0:s1 - Lq], start=True, stop=True)
                        nc.scalar.copy(qkt2[:, s0:s1], ps[:, 0:s1 - s0])

                # v projection (keys compacted: only validity col needed)
                v2 = vaug.tile([P, ntk, 130], BF16)
                for t in range(ntk):
                    ps = pssmall.tile([P, P], F32, tag="small")
                    nc.tensor.matmul(
                        out=ps[:],
                        lhsT=vts[:, c, t * P:(t + 1) * P],
                        rhs=w2s[:, c, :],
                        start=True, stop=True,
                    )
                    base = v2[:, t, 0:64]
                    vt_out = bass.AP(
                        tensor=base.tensor, offset=base.offset,
                        ap=[list(base.ap[0]), [65, 2], [1, 64]])
                    nc.vector.tensor_copy(
                        vt_out, ps[:].rearrange("p (two d) -> p two d", two=2))
                # denominator "ones" columns = slot-validity mask
                nc.vector.tensor_copy(v2[:, :, 64], kms[:, :])
                nc.vector.tensor_copy(v2[:, :, 129], kms[:, :])

                for hf in range(2):
                    h = 2 * c + hf
                    hq = qt2[64 * hf:64 * hf + 64, :]
                    hk = kt2[64 * hf:64 * hf + 64, :]
                    # scores (transposed, [k, q]) + exp -> P (bf16)
                    pt = ppool.tile([P, ntk, Lq], BF16)
                    for t in range(ntk):
                        sps = psbig.tile([P, Lq], F32, tag="big")
                        for s0, s1 in _chunks(Lq):
                            nc.tensor.matmul(
                                out=sps[:, s0:s1],
                                lhsT=hk[:, t * P:(t + 1) * P],
                                rhs=hq[:, s0:s1],
                                start=True, stop=True,
                            )
                        nc.scalar.activation(
                            out=pt[:, t, :], in_=sps[:],
                            func=mybir.ActivationFunctionType.Exp,
                            scale=0.125,
                        )
                    # PV: out[65, q] accumulated over k-tiles, wide rhs
                    pv = pspv.tile([65, Lq], F32)
                    for kt in range(ntk):
                        for s0, s1 in _chunks(Lq):
                            nc.tensor.matmul(
                                out=pv[:, s0:s1],
                                lhsT=v2[:, kt, 65 * hf:65 * hf + 65],
                                rhs=pt[:, kt, s0:s1],
                                start=(kt == 0), stop=(kt == ntk - 1),
                            )
                    # fast evacuation: denom row + unnormalized C^T rows
                    dtmp = dtpool.tile([65, Lq], F32)
                    nc.vector.tensor_copy(dtmp[64:65, :], pv[64:65, :])
                    bi = next(i for i, (a, b, _) in enumerate(NORM_BATCHES)
                              if a <= h < b)
                    hrel = h - NORM_BATCHES[bi][0]
                    nc.gpsimd.dma_start(
                        out=dstacks[bi][hrel * ntq:(hrel + 1) * ntq, :],
                        in_=dtmp[64:65, :])
                    nc.vector.tensor_copy(ct[64 * hf:64 * hf + 64, c, :], pv[0:64, :])

                for bi, (_, _, bc_) in enumerate(NORM_BATCHES):
                    if c == bc_ and bi < 2:
                        normalize_batch(bi)

            # tail: last normalize batch, then the split output projection
            normalize_batch(2)

            # part A: chunks 0-6 (normalized after batch 1) can overlap
            # the batch-2 normalize chain
            for t in range(ntq):
                yps = psbig.tile([P, E], F32, tag="big")
                final_mms(t, yps, list(range(7)))
                nc.vector.tensor_copy(ysum[:, t, :], yps[:])
            # part B: chunk 7 + combine
            for t in range(ntq):
                yps = psbig.tile([P, E], F32, tag="big")
                final_mms(t, yps, [7])
                ys = ystage.tile([P, E], F32, tag="ys")
                nc.vector.tensor_add(ys[:], yps[:], ysum[:, t, :])
                nc.gpsimd.dma_start(out=Y[t * P:(t + 1) * P, :], in_=ys[:])

    nc.compile()
    return nc


def make_core_inputs(Q, K, V, HeadLinear, OutputLiner, QMask, KMask):
    """Host-side sharding/compaction. Returns (in_maps, qidxs, ntq, ntk)."""
    bf16 = ml_dtypes.bfloat16
    qm = np.asarray(QMask).astype(bool)
    km = np.asarray(KMask).astype(bool)
    qidxs = [np.nonzero(qm[b])[0] for b in range(B)]
    kidxs = [np.nonzero(km[b])[0] for b in range(B)]
    ntq = max(1, math.ceil(max(len(ix) for ix in qidxs) / P))
    ntk = max(1, math.ceil(max(len(ix) for ix in kidxs) / P))
    Lq, Lk = ntq * P, ntk * P

    w2 = np.zeros((P, NCH, P), dtype=np.float32)
    hl = np.asarray(HeadLinear, dtype=np.float32)
    for c in range(NCH):
        w2[0:64, c, 0:64] = hl[2 * c]
        w2[64:128, c, 64:128] = hl[2 * c + 1]
    w2b = w2.astype(bf16)
    ob = np.asarray(OutputLiner, dtype=np.float32).astype(bf16)

    in_maps = []
    for b in range(B):
        qi, ki = qidxs[b], kidxs[b]
        qc = np.zeros((Lq, E), dtype=np.float32)
        qc[:len(qi)] = np.asarray(Q[b], dtype=np.float32)[qi]
        kc = np.zeros((Lk, E), dtype=np.float32)
        kc[:len(ki)] = np.asarray(K[b], dtype=np.float32)[ki]
        vc = np.zeros((Lk, E), dtype=np.float32)
        vc[:len(ki)] = np.asarray(V[b], dtype=np.float32)[ki]
        kmc = np.zeros(Lk, dtype=np.float32)
        kmc[:len(ki)] = 1.0
        in_maps.append({
            "QT": np.ascontiguousarray(qc.T.astype(bf16)),
            "KT": np.ascontiguousarray(kc.T.astype(bf16)),
            "VT": np.ascontiguousarray(vc.T.astype(bf16)),
            "W2": w2b, "OB": ob,
            "KM": np.ascontiguousarray(kmc.reshape(ntk, P).T),
        })
    return in_maps, qidxs, ntq, ntk


_NC_CACHE = {}


def _get_nc(ntq, ntk):
    if (ntq, ntk) not in _NC_CACHE:
        _NC_CACHE[(ntq, ntk)] = build_bass(ntq, ntk)
    return _NC_CACHE[(ntq, ntk)]


def kernel(Q, K, V, HeadLinear, OutputLiner, QMask, KMask):
    from concourse.bass_utils import run_bass_kernel_spmd

    in_maps, qidxs, ntq, ntk = make_core_inputs(
        Q, K, V, HeadLinear, OutputLiner, QMask, KMask)
    nc = _get_nc(ntq, ntk)
    res = run_bass_kernel_spmd(nc, in_maps, list(range(B)))
    out = np.zeros((B, L, E), dtype=np.float32)
    for b in range(B):
        yc = np.asarray(res.results[b]["Y"])
        out[b][qidxs[b]] = yc[:len(qidxs[b])]
    return out
